# revision 2
# baseline (speedup 1.0000x reference)
"""Trainium2 Bass kernel for CartNN minimal-NEAT forward pass.

Computes out = tanh(tanh(x @ w + b))[:, None] for x [16384, 4096] f32,
w [4096] f32, b [1] f32, data-parallel across 8 NeuronCores (2048 batch
rows per core). Memory-bound: each core streams its 32 MiB x shard once.

Per-core structure (measured on HW, iterated via NTFF profiles):
  - SDMA engine 15 is ~20% slower than engines 0-14 (known HW quirk:
    descriptor-ring AXI port contention). A uniform [128, 4096]x16
    tiling puts 2 MiB on every engine, so the whole stream used to end
    when engine 15 finished (~96 us busy vs ~79.5 for the rest). The
    partition->engine map is engine(p) = 2*((p%32)//4) + p//64, so
    engine 15 serves partitions {92-95, 124-127} and engine 13
    {88-91, 120-123}. Tiling as one [128, 4096] tile + sixteen
    [120, 4096] tiles (partitions 0..119) moves most of engines 13/15's
    bytes onto the 14 fast engines: 2.125 MiB each (~84.5 us) vs
    ~1.1 MiB on 13/15 — the stream no longer waits on engine 15.
  - x streams on the sync HWDGE ring, kept free of any other traffic;
    w and b load via the scalar ring (16 KiB, one descriptor — stride-0
    DRAM broadcast DMAs measurably poison the x stream, so w is
    broadcast to all 128 partitions by TensorE outer products
    ones[128,1] @ w[1,512], PSUM->SBUF copies on ScalarE).
  - The dot product is one fused mul+reduce VectorE op per tile
    (affine_mul_reduce, ~4.5 us; TensorTensorReduce crashes the
    device). The first 4 tiles are split along K with a staggered
    emission so DVE starts before the w broadcast completes. Tiles 8
    and 10 are offloaded (GpSimd multiply + ScalarE activation-accum
    reduce) so DVE keeps pace with the ~4.9 us/tile arrival rate and
    is idle, not backlogged, when the last x bytes land.
  - Output is emitted in two chunks: tiles 0-13 are tanh'd, TensorE-
    transposed and DMA'd mid-stream (completely hidden), so the
    end-of-kernel chain is only the 3-column tail: final quarter
    affines, 3 adds, tanh(tanh(.+b)), [128,3] transpose, one small
    DMA of 480-B-contiguous rows.
"""

import numpy as np

import concourse.bacc as bacc
import concourse.mybir as mybir
from concourse.bass_utils import run_bass_kernel_spmd
from concourse.masks import make_identity
from concourse.tile import TileContext

N_CORES = 8
BATCH = 16384
IN_SIZE = 4096
P = 128
PT = 120  # partitions used by engine-15-starved tiles
B_PER_CORE = BATCH // N_CORES  # 2048
N_TILES = 17  # tile 0: 128 rows; tiles 1..16: 120 rows

_NC_CACHE = None


def _row_start(t):
    return 0 if t == 0 else P + PT * (t - 1)


def _rows(t):
    return P if t == 0 else PT


def _build():
    nc = bacc.Bacc(
        "TRN2",
        target_bir_lowering=False,
        debug=False,
        num_devices=N_CORES,
    )
    x = nc.dram_tensor(
        "x", [B_PER_CORE, IN_SIZE], mybir.dt.float32, kind="ExternalInput"
    )
    w = nc.dram_tensor("w", [IN_SIZE], mybir.dt.float32, kind="ExternalInput")
    b = nc.dram_tensor("b", [1], mybir.dt.float32, kind="ExternalInput")
    y = nc.dram_tensor("y", [B_PER_CORE, 1], mybir.dt.float32, kind="ExternalOutput")

    # y views: y[0:128] (tile 0, 512 B) and y[128:2048] as [16, 120]
    # (tiles 1..16, 480-B-contiguous rows).
    yv_head = y.rearrange("(a p) o -> a (p o)", p=P)  # [16, 128]; row 0 used
    yv_rest = y[P:B_PER_CORE].rearrange("(t p) o -> t (p o)", p=PT)  # [16, 120]

    N_A = 14  # tiles 0..13 emitted mid-stream; 14..16 at the end

    with TileContext(nc) as tc:
        with (
            tc.tile_pool(name="xpool", bufs=8) as xpool,
            tc.tile_pool(name="scratch", bufs=1) as spool,
            tc.tile_pool(name="consts", bufs=1) as cpool,
            tc.tile_pool(name="psum", bufs=1, space="PSUM") as ppool,
        ):
            # w/b arrive on the scalar HWDGE ring so the sync ring carries
            # nothing but the x stream. TensorE broadcasts w to all 128
            # partitions chunk by chunk (ones[128,1] @ w[1,512] outer
            # products, copied PSUM->SBUF by the otherwise-idle ScalarE).
            w_1K = cpool.tile([1, IN_SIZE], mybir.dt.float32)
            nc.scalar.dma_start(out=w_1K[:], in_=w[None, :])
            b_11 = cpool.tile([1, 1], mybir.dt.float32)
            nc.scalar.dma_start(out=b_11[:], in_=b[None, :])
            ones_1P = cpool.tile([1, P], mybir.dt.float32)
            nc.vector.memset(ones_1P[:], 1.0)

            # Accumulators. Partitions 120-127 of the [120]-tile columns
            # are never written by compute; memset once so every later
            # read (tanh, folds) has a defined writer.
            acc_PT = cpool.tile([P, N_TILES], mybir.dt.float32)
            nc.vector.memset(acc_PT[:], 0.0)
            NSPLIT = 4
            NQT = 4  # tiles that use the quarter-split
            STAGGER = 3
            KQ = IN_SIZE // NSPLIT
            accs_q = [
                cpool.tile([P, NQT], mybir.dt.float32, name=f"acc_{q}")
                for q in range(1, NSPLIT)
            ]
            for acc_q in accs_q:
                nc.vector.memset(acc_q[:], 0.0)
            acc_last = cpool.tile([P, 6], mybir.dt.float32)
            nc.vector.memset(acc_last[:], 0.0)

            w_PK = cpool.tile([P, IN_SIZE], mybir.dt.float32)
            NCHUNK = 512
            for c in range(IN_SIZE // NCHUNK):
                cs = slice(c * NCHUNK, (c + 1) * NCHUNK)
                w_psum = ppool.tile([P, NCHUNK], mybir.dt.float32, bufs=2)
                nc.tensor.matmul(w_psum[:], ones_1P[:], w_1K[0:1, cs])
                nc.scalar.copy(w_PK[:, cs], w_psum[:])
            b_psum = ppool.tile([P, 1], mybir.dt.float32)
            nc.tensor.matmul(b_psum[:], ones_1P[:], b_11[:])
            b_P1 = cpool.tile([P, 1], mybir.dt.float32)
            nc.scalar.copy(b_P1[:], b_psum[:])
            ident = cpool.tile([P, P], mybir.dt.float32)
            make_identity(nc, ident[:])

            prod_PK = spool.tile([P, IN_SIZE], mybir.dt.float32)
            x_tiles = {}

            def load_x(t):
                x_PK = xpool.tile([P, IN_SIZE], mybir.dt.float32)
                r = _rows(t)
                r0 = _row_start(t)
                nc.sync.dma_start(out=x_PK[0:r, :], in_=x[r0 : r0 + r, :])
                x_tiles[t] = x_PK

            # The first 4 tiles are split into quarter-K ops with a
            # staggered emission (quarter q of tile t at step t + 3q):
            # quarter q only needs w[q*1024:(q+1)*1024], so DVE starts as
            # soon as the first w chunks are broadcast instead of waiting
            # for all of w. The Tile scheduler keeps same-engine program
            # order, so the stagger must be explicit.
            def emit_quarter(t, q):
                seg = slice(q * KQ, (q + 1) * KQ)
                r = _rows(t)
                acc = acc_PT[:, t : t + 1] if q == 0 else accs_q[q - 1][:, t : t + 1]
                nc.vector.affine_mul_reduce(
                    out=prod_PK[0:r, seg],
                    accum_out=acc[0:r],
                    in0=x_tiles[t][0:r, seg],
                    in1=w_PK[0:r, seg],
                    scale=1.0,
                    bias=0.0,
                )

            for i in range(NQT + STAGGER * (NSPLIT - 1)):
                if i < NQT:
                    load_x(i)
                    emit_quarter(i, 0)
                for q in range(1, NSPLIT):
                    t = i - STAGGER * q
                    if 0 <= t < NQT:
                        emit_quarter(t, q)
            # Fold the quarter partials early (DVE program order!) so the
            # mid-stream chunk-A output only waits on tile 13's affine.
            for acc_q in accs_q:
                nc.vector.tensor_add(acc_PT[:, 0:NQT], acc_PT[:, 0:NQT], acc_q[:])

            # Mid tiles. Two are offloaded off the (binding) VectorE:
            # GpSimd does the elementwise multiply, ScalarE reduces it via
            # activation-accum. Both engines are otherwise idle mid-kernel.
            # Tiles 8/10: their x-ring slots (0/2) are only reused by tile
            # 16's load, long after GpSimd's ~11 us hold ends.
            GPS_TILES = (8, 10)
            prod2_PK = spool.tile(
                [P, IN_SIZE], mybir.dt.float32, name="prod2_PK", tag="prod2"
            )
            for t in range(NQT, N_TILES - 2):
                load_x(t)
                r = _rows(t)
                if t in GPS_TILES:
                    nc.gpsimd.tensor_mul(
                        prod2_PK[0:r, :], x_tiles[t][0:r, :], w_PK[0:r, :]
                    )
                    nc.scalar.activation(
                        prod2_PK[0:r, :],
                        prod2_PK[0:r, :],
                        mybir.ActivationFunctionType.Copy,
                        accum_out=acc_PT[0:r, t : t + 1],
                    )
                    continue
                nc.vector.affine_mul_reduce(
                    out=prod_PK[0:r, :],
                    accum_out=acc_PT[0:r, t : t + 1],
                    in0=x_tiles[t][0:r, :],
                    in1=w_PK[0:r, :],
                    scale=1.0,
                    bias=0.0,
                )

            # Chunk A: tiles 0..13 go tanh -> transpose -> DMA as soon as
            # tile 13's accumulate lands (~83 us), fully hidden under the
            # tail of the x stream. No DVE ops here — ScalarE/TensorE only.
            y_A = cpool.tile([P, N_A], mybir.dt.float32)
            nc.scalar.activation(
                y_A[:],
                acc_PT[:, 0:N_A],
                mybir.ActivationFunctionType.Tanh,
                bias=b_P1[:],
            )
            nc.scalar.activation(y_A[:], y_A[:], mybir.ActivationFunctionType.Tanh)
            yps_A = ppool.tile([N_A, P], mybir.dt.float32)
            nc.tensor.transpose(yps_A[:], y_A[:], ident[:])
            y_TA = cpool.tile([N_A, P], mybir.dt.float32)
            nc.scalar.copy(y_TA[:], yps_A[:])
            nc.scalar.dma_start(out=yv_rest[0 : N_A - 1], in_=y_TA[1:N_A, 0:PT])
            nc.scalar.dma_start(out=yv_head[0:1], in_=y_TA[0:1, :])

            # The last two tiles are split (loads AND compute: halves for
            # t=15, quarters for t=16) so the final compute piece starts
            # on the last ~470 KiB rather than the last 1.9 MiB.
            def split_tile(t, nsplit, acc_off):
                seg_k = IN_SIZE // nsplit
                r = _rows(t)
                r0 = _row_start(t)
                x_PK = xpool.tile([P, IN_SIZE], mybir.dt.float32)
                x_tiles[t] = x_PK
                for s in range(nsplit):
                    seg = slice(s * seg_k, (s + 1) * seg_k)
                    nc.sync.dma_start(
                        out=x_PK[0:r, seg], in_=x[r0 : r0 + r, seg]
                    )
                    nc.vector.affine_mul_reduce(
                        out=prod_PK[0:r, seg],
                        accum_out=acc_last[0:r, acc_off + s : acc_off + s + 1],
                        in0=x_PK[0:r, seg],
                        in1=w_PK[0:r, seg],
                        scale=1.0,
                        bias=0.0,
                    )

            split_tile(N_TILES - 2, 2, 0)
            split_tile(N_TILES - 1, 4, 2)

            # Combine the split partial sums of tiles 15/16.
            t15, t16 = N_TILES - 2, N_TILES - 1
            nc.vector.tensor_add(
                acc_PT[:, t15 : t15 + 1], acc_last[:, 0:1], acc_last[:, 1:2]
            )
            nc.vector.tensor_add(
                acc_last[:, 2:4], acc_last[:, 2:4], acc_last[:, 4:6]
            )
            nc.vector.tensor_add(
                acc_PT[:, t16 : t16 + 1], acc_last[:, 2:3], acc_last[:, 3:4]
            )

            # Chunk B: only the 3-column tail. tanh(tanh(acc + b)) on
            # ScalarE (the DVE->ACT handoff needs no DVE drain), TensorE
            # transpose [128, 3] -> [3, 128], one 1.4 KiB DMA of 480-B
            # rows from the scalar ring (ScalarE just wrote y_TB, so this
            # skips the ScalarE->Sync semaphore hop, and the sync
            # sequencer is still busy with x-load completions).
            N_B = N_TILES - N_A
            y_B = cpool.tile([P, N_B], mybir.dt.float32)
            nc.scalar.activation(
                y_B[:],
                acc_PT[:, N_A:N_TILES],
                mybir.ActivationFunctionType.Tanh,
                bias=b_P1[:],
            )
            nc.scalar.activation(y_B[:], y_B[:], mybir.ActivationFunctionType.Tanh)
            yps_B = ppool.tile([N_B, P], mybir.dt.float32)
            nc.tensor.transpose(yps_B[:], y_B[:], ident[:])
            y_TB = cpool.tile([N_B, P], mybir.dt.float32)
            nc.scalar.copy(y_TB[:], yps_B[:])
            nc.scalar.dma_start(out=yv_rest[N_A - 1 : 16], in_=y_TB[:, 0:PT])
    nc.compile()
    return nc


def _get_nc():
    global _NC_CACHE
    if _NC_CACHE is None:
        _NC_CACHE = _build()
    return _NC_CACHE


def _run(x, w, b, **spmd_kwargs):
    """Shard, execute on 8 cores, gather. Returns (out, BassKernelResults)."""
    x = np.ascontiguousarray(np.asarray(x, dtype=np.float32))
    w = np.ascontiguousarray(np.asarray(w, dtype=np.float32))
    b = np.ascontiguousarray(np.asarray(b, dtype=np.float32))
    assert x.shape == (BATCH, IN_SIZE), x.shape

    nc = _get_nc()
    in_maps = [
        {"x": x[c * B_PER_CORE : (c + 1) * B_PER_CORE], "w": w, "b": b}
        for c in range(N_CORES)
    ]
    res = run_bass_kernel_spmd(nc, in_maps, list(range(N_CORES)), **spmd_kwargs)
    out = np.concatenate(
        [np.asarray(res.results[c]["y"]) for c in range(N_CORES)], axis=0
    )
    return out.astype(np.float32, copy=False), res


def kernel(x, w, b):
    try:
        out, _ = _run(x, w, b)
    except Exception:
        # Transient device-wedge (NRT_EXEC_UNIT_UNRECOVERABLE) has been
        # observed once on a first run and succeeded on retry.
        out, _ = _run(x, w, b)
    return out


# revision 3
# speedup vs baseline: 1.5073x; 1.5073x over previous
"""Trainium2 Bass kernel for CartNN minimal-NEAT forward pass.

Computes out = tanh(tanh(x @ w + b))[:, None] for x [16384, 4096] f32,
w [4096] f32, b [1] f32, data-parallel across 8 NeuronCores (2048 batch
rows per core). Memory-bound: each core streams its 32 MiB x shard once.

Per-core structure (measured on HW, iterated via NTFF profiles):
  - x streams as 16 [128, 4096] tiles on the sync HWDGE ring, issued
    before anything else so the stream starts at ~5.7 us instead of
    ~8.7 (w/b load via the scalar ring). All x DMAs write full
    128-partition tiles: partition-sliced destinations (e.g. [0:120])
    measurably halve the per-descriptor SDMA rate (port-swizzle
    misalignment), which is also why engine-15 starvation via 120-row
    tiles regressed 117 -> 175 us.
  - SDMA engine 15 is ~20% slower than engines 0-14 (known HW quirk),
    and with uniform tiles it carries 1/16 of the bytes, so the
    stream is engine-15-bound: ~96 us busy vs ~79.5 for the rest.
  - w is loaded once (16 KiB, scalar ring) and broadcast to all 128
    partitions by TensorE outer products ones[128,1] @ w[1,512]
    (PSUM->SBUF copies on ScalarE): zero extra HBM traffic and no
    sync-ring involvement (stride-0 DRAM broadcast DMAs measurably
    poison the x stream).
  - The dot product is one fused mul+reduce VectorE op per tile
    (affine_mul_reduce, ~4.5 us; TensorTensorReduce crashes the
    device). The first 4 tiles are split along K with staggered
    emission so DVE starts before the w broadcast completes; their
    quarter-partials are folded into acc right after the stagger (DVE
    program order) so the mid-stream output chunk doesn't wait.
  - Tiles 8 and 10 are offloaded off VectorE: GpSimd multiplies,
    ScalarE reduces via activation-accum. With the ~6 us/tile
    engine-15-paced arrival rate DVE then idles between tiles instead
    of being backlogged when the last x bytes land.
  - Output is emitted in two chunks: tiles 0..13 go tanh(tanh(.+b)) ->
    TensorE transpose -> [14,128] DMA mid-stream (fully hidden under
    the x stream tail); the end-of-kernel chain is only tiles 14/15:
    final quarter affines, 3 adds, tanh x2 on [128,2], transpose, one
    1 KiB DMA of 512-B rows from the scalar ring (ScalarE just wrote
    the data, skipping the ScalarE->Sync semaphore hop).
  - The last two tiles are split (loads AND compute: halves for t=14,
    quarters for t=15) so the final compute piece starts on the last
    512 KiB rather than the last 2 MiB.
"""

import numpy as np

import concourse.bacc as bacc
import concourse.mybir as mybir
from concourse.bass_utils import run_bass_kernel_spmd
from concourse.masks import make_identity
from concourse.tile import TileContext

N_CORES = 8
BATCH = 16384
IN_SIZE = 4096
P = 128
B_PER_CORE = BATCH // N_CORES  # 2048
N_TILES = B_PER_CORE // P  # 16

_NC_CACHE = None


def _build():
    nc = bacc.Bacc(
        "TRN2",
        target_bir_lowering=False,
        debug=False,
        num_devices=N_CORES,
    )
    x = nc.dram_tensor(
        "x", [B_PER_CORE, IN_SIZE], mybir.dt.float32, kind="ExternalInput"
    )
    w = nc.dram_tensor("w", [IN_SIZE], mybir.dt.float32, kind="ExternalInput")
    b = nc.dram_tensor("b", [1], mybir.dt.float32, kind="ExternalInput")
    y = nc.dram_tensor("y", [B_PER_CORE, 1], mybir.dt.float32, kind="ExternalOutput")

    xt = x.rearrange("(t p) k -> t p k", p=P)  # [16, 128, 4096]
    yv = y.rearrange("(t p) o -> t (p o)", p=P)  # [16, 128], 512B rows

    N_A = 14  # tiles 0..13 emitted mid-stream; 14..15 at the end
    N_B = N_TILES - N_A

    with TileContext(nc) as tc:
        with (
            tc.tile_pool(name="xpool", bufs=8) as xpool,
            tc.tile_pool(name="scratch", bufs=1) as spool,
            tc.tile_pool(name="consts", bufs=1) as cpool,
            tc.tile_pool(name="psum", bufs=1, space="PSUM") as ppool,
        ):
            x_tiles = {}

            def load_x(t):
                x_PK = xpool.tile([P, IN_SIZE], mybir.dt.float32)
                nc.sync.dma_start(out=x_PK[:], in_=xt[t])
                x_tiles[t] = x_PK

            # Tile 0's load is the very first sync-ring op so the x
            # stream starts as early as the NEFF preamble allows.
            load_x(0)

            # w/b arrive on the scalar HWDGE ring, keeping the sync ring
            # clear for the stream. TensorE broadcasts w to all 128
            # partitions chunk by chunk: ones[128,1] @ w[1,512] outer
            # products, copied PSUM->SBUF by the otherwise-idle ScalarE.
            w_1K = cpool.tile([1, IN_SIZE], mybir.dt.float32)
            nc.scalar.dma_start(out=w_1K[:], in_=w[None, :])
            b_11 = cpool.tile([1, 1], mybir.dt.float32)
            nc.scalar.dma_start(out=b_11[:], in_=b[None, :])
            ones_1P = cpool.tile([1, P], mybir.dt.float32)
            nc.vector.memset(ones_1P[:], 1.0)

            acc_PT = cpool.tile([P, N_TILES], mybir.dt.float32)
            NSPLIT = 4
            NQT = 4  # tiles that use the quarter-split
            STAGGER = 3
            KQ = IN_SIZE // NSPLIT
            accs_q = [
                cpool.tile([P, NQT], mybir.dt.float32, name=f"acc_{q}")
                for q in range(1, NSPLIT)
            ]
            acc_last = cpool.tile([P, 6], mybir.dt.float32)

            w_PK = cpool.tile([P, IN_SIZE], mybir.dt.float32)
            NCHUNK = 512
            for c in range(IN_SIZE // NCHUNK):
                cs = slice(c * NCHUNK, (c + 1) * NCHUNK)
                w_psum = ppool.tile([P, NCHUNK], mybir.dt.float32, bufs=2)
                nc.tensor.matmul(w_psum[:], ones_1P[:], w_1K[0:1, cs])
                nc.scalar.copy(w_PK[:, cs], w_psum[:])
            b_psum = ppool.tile([P, 1], mybir.dt.float32)
            nc.tensor.matmul(b_psum[:], ones_1P[:], b_11[:])
            b_P1 = cpool.tile([P, 1], mybir.dt.float32)
            nc.scalar.copy(b_P1[:], b_psum[:])
            ident = cpool.tile([P, P], mybir.dt.float32)
            make_identity(nc, ident[:])

            prod_PK = spool.tile([P, IN_SIZE], mybir.dt.float32)

            # The first 4 tiles are split into quarter-K ops with a
            # staggered emission (quarter q of tile t at step t + 3q):
            # quarter q only needs w[q*1024:(q+1)*1024], so DVE starts as
            # soon as the first w chunks are broadcast instead of waiting
            # for all of w. The Tile scheduler keeps same-engine program
            # order, so the stagger must be explicit.
            def emit_quarter(t, q):
                seg = slice(q * KQ, (q + 1) * KQ)
                acc = acc_PT[:, t : t + 1] if q == 0 else accs_q[q - 1][:, t : t + 1]
                nc.vector.affine_mul_reduce(
                    out=prod_PK[:, seg],
                    accum_out=acc,
                    in0=x_tiles[t][:, seg],
                    in1=w_PK[:, seg],
                    scale=1.0,
                    bias=0.0,
                )

            for i in range(NQT + STAGGER * (NSPLIT - 1)):
                if 0 < i < NQT:
                    load_x(i)
                if i < NQT:
                    emit_quarter(i, 0)
                for q in range(1, NSPLIT):
                    t = i - STAGGER * q
                    if 0 <= t < NQT:
                        emit_quarter(t, q)
            # Fold the quarter partials early (DVE program order!) so the
            # mid-stream chunk-A output only waits on tile 13's affine.
            for acc_q in accs_q:
                nc.vector.tensor_add(acc_PT[:, 0:NQT], acc_PT[:, 0:NQT], acc_q[:])

            # Mid tiles. Two are offloaded off the (binding) VectorE:
            # GpSimd does the elementwise multiply, ScalarE reduces it via
            # activation-accum. Both engines are otherwise idle mid-kernel
            # and finish long before their results are needed. The
            # offloaded tiles MUST be >= 8: with an 8-buffer x ring, slots
            # of tiles 8..15 are never reused, so GpSimd's ~11 us hold of
            # its x tile cannot block a later load.
            GPS_TILES = (8, 10)
            prod2_PK = spool.tile(
                [P, IN_SIZE], mybir.dt.float32, name="prod2_PK", tag="prod2"
            )
            for t in range(NQT, N_TILES - 2):
                load_x(t)
                if t in GPS_TILES:
                    nc.gpsimd.tensor_mul(prod2_PK[:], x_tiles[t][:], w_PK[:])
                    nc.scalar.activation(
                        prod2_PK[:],
                        prod2_PK[:],
                        mybir.ActivationFunctionType.Copy,
                        accum_out=acc_PT[:, t : t + 1],
                    )
                    continue
                nc.vector.affine_mul_reduce(
                    out=prod_PK[:],
                    accum_out=acc_PT[:, t : t + 1],
                    in0=x_tiles[t][:],
                    in1=w_PK[:],
                    scale=1.0,
                    bias=0.0,
                )

            # Chunk A: tiles 0..13 go tanh -> transpose -> DMA as soon as
            # tile 13's accumulate lands, fully hidden under the tail of
            # the x stream. No DVE ops here — ScalarE/TensorE only.
            y_A = cpool.tile([P, N_A], mybir.dt.float32)
            nc.scalar.activation(
                y_A[:],
                acc_PT[:, 0:N_A],
                mybir.ActivationFunctionType.Tanh,
                bias=b_P1[:],
            )
            nc.scalar.activation(y_A[:], y_A[:], mybir.ActivationFunctionType.Tanh)
            yps_A = ppool.tile([N_A, P], mybir.dt.float32)
            nc.tensor.transpose(yps_A[:], y_A[:], ident[:])
            y_TA = cpool.tile([N_A, P], mybir.dt.float32)
            nc.scalar.copy(y_TA[:], yps_A[:])
            nc.scalar.dma_start(out=yv[0:N_A], in_=y_TA[:])

            # The last two tiles are split (loads AND compute: halves for
            # t=14, quarters for t=15) so the final compute piece starts
            # on the last 512 KiB rather than the last 2 MiB.
            def split_tile(t, nsplit, acc_off):
                seg_k = IN_SIZE // nsplit
                x_PK = xpool.tile([P, IN_SIZE], mybir.dt.float32)
                x_tiles[t] = x_PK
                for s in range(nsplit):
                    seg = slice(s * seg_k, (s + 1) * seg_k)
                    nc.sync.dma_start(out=x_PK[:, seg], in_=xt[t][:, seg])
                    nc.vector.affine_mul_reduce(
                        out=prod_PK[:, seg],
                        accum_out=acc_last[:, acc_off + s : acc_off + s + 1],
                        in0=x_PK[:, seg],
                        in1=w_PK[:, seg],
                        scale=1.0,
                        bias=0.0,
                    )

            split_tile(N_TILES - 2, 2, 0)
            split_tile(N_TILES - 1, 4, 2)

            # Combine the split partial sums of tiles 14/15.
            t14, t15 = N_TILES - 2, N_TILES - 1
            nc.vector.tensor_add(
                acc_PT[:, t14 : t14 + 1], acc_last[:, 0:1], acc_last[:, 1:2]
            )
            nc.vector.tensor_add(
                acc_last[:, 2:4], acc_last[:, 2:4], acc_last[:, 4:6]
            )
            nc.vector.tensor_add(
                acc_PT[:, t15 : t15 + 1], acc_last[:, 2:3], acc_last[:, 3:4]
            )

            # Chunk B: only the 2-column tail. tanh(tanh(acc + b)) on
            # ScalarE (the DVE->ACT handoff needs no DVE drain), TensorE
            # transpose [128, 2] -> [2, 128], one 1 KiB DMA of 512-B rows
            # from the scalar ring.
            y_B = cpool.tile([P, N_B], mybir.dt.float32)
            nc.scalar.activation(
                y_B[:],
                acc_PT[:, N_A:N_TILES],
                mybir.ActivationFunctionType.Tanh,
                bias=b_P1[:],
            )
            nc.scalar.activation(y_B[:], y_B[:], mybir.ActivationFunctionType.Tanh)
            yps_B = ppool.tile([N_B, P], mybir.dt.float32)
            nc.tensor.transpose(yps_B[:], y_B[:], ident[:])
            y_TB = cpool.tile([N_B, P], mybir.dt.float32)
            nc.scalar.copy(y_TB[:], yps_B[:])
            nc.scalar.dma_start(out=yv[N_A:N_TILES], in_=y_TB[:])
    nc.compile()
    return nc


def _get_nc():
    global _NC_CACHE
    if _NC_CACHE is None:
        _NC_CACHE = _build()
    return _NC_CACHE


def _run(x, w, b, **spmd_kwargs):
    """Shard, execute on 8 cores, gather. Returns (out, BassKernelResults)."""
    x = np.ascontiguousarray(np.asarray(x, dtype=np.float32))
    w = np.ascontiguousarray(np.asarray(w, dtype=np.float32))
    b = np.ascontiguousarray(np.asarray(b, dtype=np.float32))
    assert x.shape == (BATCH, IN_SIZE), x.shape

    nc = _get_nc()
    in_maps = [
        {"x": x[c * B_PER_CORE : (c + 1) * B_PER_CORE], "w": w, "b": b}
        for c in range(N_CORES)
    ]
    res = run_bass_kernel_spmd(nc, in_maps, list(range(N_CORES)), **spmd_kwargs)
    out = np.concatenate(
        [np.asarray(res.results[c]["y"]) for c in range(N_CORES)], axis=0
    )
    return out.astype(np.float32, copy=False), res


def kernel(x, w, b):
    try:
        out, _ = _run(x, w, b)
    except Exception:
        # Transient device-wedge (NRT_EXEC_UNIT_UNRECOVERABLE) has been
        # observed once on a first run and succeeded on retry.
        out, _ = _run(x, w, b)
    return out
